# revision 34
# baseline (speedup 1.0000x reference)
"""Trainium2 Bass kernel for the DND retrieval problem.

Full (unsharded) inputs in, full output out. Data-parallel over batch B=64
across 8 NeuronCores (8 batch elements per core), one SPMD Bass program.

Design notes:
- Every large operand ships as fp16 (half the HBM bytes of f32; fp16's
  10-bit mantissa keeps absmax rel err ~3e-3, well under the 2e-2 gate).
- keys are pre-transposed on the host to [k, l] layout so no on-chip
  transposes are needed for the scores matmuls.
- All tensors are SBUF-resident, so every DMA is issued eagerly with no
  waits and the DMA engines stream back-to-back.
- Softmax weights are exactly zero for l >= steps[b], so those (l, b)
  slices of keys/vals are never needed. The host sorts batches by steps
  (descending) and deals them round-robin across cores, so per l-chunk the
  live batches are a prefix and all cores share one live-count profile;
  the program is specialized to that profile (cached per profile) and only
  streams/computes the live prefix of each chunk (~35% fewer bytes for
  uniform steps).

Per-core program (Bc=8, H=8, K=V=256, L=2048):
  qTblk[k, 64]   block-diagonal q (col 8b+h) from wq/query (PE prologue)
  scoresT[bh, l] 2*nb fp16 matmuls per 256-l chunk into a [64,2048] PSUM tile
  softmax over l: scoresT = pscore*rpeT + addmask; global max; ACT Exp with
                  accum rowsums; recip folded back into scoresT
  readT[v, bh]   per 128-l chunk: PE transpose of w + 2*nb tiny (N=8)
                  matmuls accumulating in PSUM across all chunks
  out[b, :]      readT.T @ Wa + ba  (un-permuted on the host)
"""

import numpy as np

import concourse.bacc as bacc
import concourse.bass as bass
import concourse.mybir as mybir
import concourse.tile as tile
from concourse.bass_utils import run_bass_kernel_spmd
from concourse.masks import make_identity
F32 = mybir.dt.float32
F16 = mybir.dt.float16

L = 2048
B = 64
K = 256
V = 256
H = 8
NCORES = 8
BC = B // NCORES          # 8 batch elements per core
NKC = 8                   # keys chunks (256 l each)
KCL = L // NKC            # 256
NVC = 16                  # vals chunks (128 l each)
VCL = L // NVC            # 128
NEG = -1.0e30


def _ap(tensor_ap, offset_elems, dims):
    """Build a raw AP on the same tensor with explicit [step, count] dims."""
    return bass.AP(tensor=tensor_ap.tensor, offset=offset_elems, ap=dims)


def _plan(steps):
    """Sort batches by steps desc, deal round-robin to cores; live-count
    profiles per chunk (max across cores, so one SPMD program fits all)."""
    steps = np.asarray(steps)
    perm = np.argsort(-steps, kind="stable")
    core_idx = [perm[np.arange(BC) * NCORES + c] for c in range(NCORES)]
    nbk = [0] * NKC
    nbv = [0] * NVC
    for c in range(NCORES):
        sc = steps[core_idx[c]]
        for ch in range(NKC):
            nbk[ch] = max(nbk[ch], int((sc > ch * KCL).sum()))
        for vc in range(NVC):
            nbv[vc] = max(nbv[vc], int((sc > vc * VCL).sum()))
    # per batch slot: number of live l-partitions in its LAST live chunk
    # (the rest of that chunk has zero weight and is never loaded/computed)
    rend = [VCL] * BC
    for b in range(BC):
        lvc = max(vc for vc in range(NVC) if nbv[vc] > b)
        r = 1
        for c in range(NCORES):
            s = int(steps[core_idx[c][b]])
            r = max(r, min(s - lvc * VCL, VCL))
        rend[b] = r
    return core_idx, tuple(nbk), tuple(nbv), tuple(rend)


def build_nc(nbk, nbv, rend):
    nc = bacc.Bacc("TRN2", target_bir_lowering=False)

    t_query = nc.dram_tensor("query", [BC, K], F32, kind="ExternalInput").ap()
    t_keysT = nc.dram_tensor("keysT", [NKC, 128, 2, BC, KCL], F16,
                             kind="ExternalInput").ap()
    t_vals = nc.dram_tensor("vals", [NVC, VCL, BC, V], F16,
                            kind="ExternalInput").ap()
    t_rpeT = nc.dram_tensor("rpeT", [B, L], F16, kind="ExternalInput").ap()
    t_wq = nc.dram_tensor("wq", [K, H * K], F16, kind="ExternalInput").ap()
    t_bq = nc.dram_tensor("bq", [H * K], F32, kind="ExternalInput").ap()
    t_wa = nc.dram_tensor("wa", [H * V, V], F16, kind="ExternalInput").ap()
    t_ba = nc.dram_tensor("ba", [V], F32, kind="ExternalInput").ap()
    t_steps = nc.dram_tensor("stepsf", [B], F32, kind="ExternalInput").ap()
    t_out = nc.dram_tensor("out", [BC, V], F32, kind="ExternalOutput").ap()

    with tile.TileContext(nc) as tc:
        _emit(nc, tc, t_query, t_keysT, t_vals, t_rpeT, t_wq, t_bq, t_wa,
              t_ba, t_steps, t_out, nbk, nbv, rend)
    nc.compile()
    return nc


def _emit(nc, tc, t_query, t_keysT, t_vals, t_rpeT, t_wq, t_bq, t_wa, t_ba,
          t_steps, t_out, nbk, nbv, rend):
    from contextlib import ExitStack
    ctx = ExitStack()
    with ctx:
        consts = ctx.enter_context(tc.tile_pool(name="consts", bufs=1))
        keysP = ctx.enter_context(tc.tile_pool(name="keysP", bufs=NKC))
        valsP = ctx.enter_context(tc.tile_pool(name="valsP", bufs=NVC))
        wsbP = ctx.enter_context(tc.tile_pool(name="wsbP", bufs=NVC))
        # PSUM budget is 8 banks: bigP (1) is time-shared by the prologue
        # transposes/q-build and po; pscP (2) double-buffers per-chunk score
        # accumulators so chunk N+1's matmuls don't wait on chunk N's DVE
        # copyback; pwP (3) pipelines the w transposes; prP (2) holds the two
        # readT accumulators (one bank per vh half — interleaved accumulation
        # groups must not share a bank).
        bigP = ctx.enter_context(tc.tile_pool(name="bigP", bufs=1, space="PSUM"))
        pscP = ctx.enter_context(tc.tile_pool(name="pscP", bufs=2, space="PSUM"))
        pwP = ctx.enter_context(tc.tile_pool(name="pwP", bufs=3, space="PSUM"))
        prP = ctx.enter_context(tc.tile_pool(name="prP", bufs=2, space="PSUM"))

        # ------------- DMA issue -------------
        # One deterministic stream on the SP/HWDGE queue (FIFO on the DMA
        # engines): wmat -> rpe -> keys -> ba/wa -> vals. The softmax chain
        # hangs off the LAST keys chunk, so keys go as early as possible;
        # ba/wa hide inside the 15us vals stream; total DMA time is fixed by
        # bytes, only the ordering of the tail matters. Tiny loads ride the
        # Activation HWDGE queue; Pool only builds ident/iota, so the PE
        # prologue is ready before the first keys chunk lands.
        wmat = consts.tile([128, 2, H * K], F16, tag="wmat")
        nc.sync.dma_start(out=wmat, in_=t_wq.rearrange("(a p) j -> p a j", a=2))
        rpeT = consts.tile([64, L], F16, tag="rpeT")
        nc.sync.dma_start(out=rpeT, in_=t_rpeT)

        keys_tiles = []
        for ch in range(NKC):
            nb = nbk[ch]
            if nb == 0:
                keys_tiles.append(None)
                continue
            kt = keysP.tile([128, 2, nb, KCL], F16, tag="keys")
            nc.sync.dma_start(out=kt, in_=t_keysT[ch][:, :, :nb, :])
            keys_tiles.append(kt)

        nbv_next = list(nbv[1:]) + [0]
        vals_tiles = []
        for vc in range(NVC):
            nb = nbv[vc]
            if nb == 0:
                vals_tiles.append(None)
                continue
            nb_full = nbv_next[vc]
            vt = valsP.tile([VCL, nb, V], F16, tag="vals")
            if nb_full > 0:
                nc.sync.dma_start(out=vt[:, :nb_full, :],
                                  in_=t_vals[vc][:, :nb_full, :])
            for b in range(nb_full, nb):
                # partial chunks ride the ACT queue: the SP queue's serial
                # issue rate would otherwise starve the stream tail
                r = rend[b]
                nc.gpsimd.dma_start(out=vt[:r, b:b + 1, :],
                                     in_=t_vals[vc][:r, b:b + 1, :])
            vals_tiles.append(vt)

        # wa/ba stream AFTER vals: they are the latest-needed operands (final
        # projection), so the end-of-stream sem latency lands on them instead
        # of the read-path vals chunks; wa is split so the first half's
        # projection matmuls overlap the second half's transfer
        wa_sb = consts.tile([128, 16, V], F16, tag="wa_sb")
        nc.sync.dma_start(
            out=wa_sb[:, :5, :],
            in_=t_wa.rearrange("(a p) j -> p a j", a=16)[:, :5, :])
        ba_rep = consts.tile([BC, V], F32, tag="ba_rep")
        nc.sync.dma_start(out=ba_rep, in_=_ap(t_ba, 0, [[0, BC], [1, V]]))
        for lo, hi in ((5, 10), (10, 15), (15, 16)):
            nc.sync.dma_start(
                out=wa_sb[:, lo:hi, :],
                in_=t_wa.rearrange("(a p) j -> p a j", a=16)[:, lo:hi, :])

        query_sb = consts.tile([BC, K], F32, tag="query")
        nc.scalar.dma_start(out=query_sb, in_=t_query)
        bq_nat = consts.tile([16, 128], F32, tag="bq_nat")
        nc.scalar.dma_start(out=bq_nat, in_=t_bq.rearrange("(r q) -> r q", r=16))
        stepsf = consts.tile([64, 1], F32, tag="stepsf")
        nc.scalar.dma_start(out=stepsf, in_=_ap(t_steps, 0, [[1, 64], [0, 1]]))
        ident = consts.tile([128, 128], F32, tag="ident")
        make_identity(nc, ident)
        iota = consts.tile([64, L], F32, tag="iota")
        nc.gpsimd.iota(iota, pattern=[[1, L]], base=0, channel_multiplier=0,
                       allow_small_or_imprecise_dtypes=True)
        # ------------- prologue compute -------------
        # scoresT starts at NEG: chunks/rows beyond the live prefix are never
        # written by the mults below and must read as fully-masked scores
        scoresT = consts.tile([64, L], F32, tag="scoresT")
        nc.vector.memset(scoresT, NEG)

        # queryT [k, b] (fp16) via PE transpose of query [b, k]
        queryT = consts.tile([128, 2, BC], F16, tag="queryT")
        for half in range(2):
            pq = bigP.tile([128, 256], F32, tag="big")
            nc.tensor.transpose(
                pq[:, :BC], query_sb[:, half * 128:(half + 1) * 128],
                ident[:BC, :BC])
            nc.any.tensor_copy(queryT[:, half, :], pq[:, :BC])

        # bqT [kout, (h,kc)] via PE transpose
        bq_sb = consts.tile([128, 16], F32, tag="bq_sb")
        pb = bigP.tile([128, 256], F32, tag="big")
        nc.tensor.transpose(pb[:, :16], bq_nat, ident[:16, :16])
        nc.any.tensor_copy(bq_sb, pb[:, :16])

        # block-diagonal qT: [kout(128), kc, b, 64 cols]; col 8b+h holds
        # q[b,h,kout], other columns zero, so one matmul per (kc, b)
        # accumulates all 64 (b,h) score rows without cross-terms
        qTblks = []
        for kc in range(2):
            qTblk = consts.tile([128, BC, 64], F16, tag=f"qTblk{kc}",
                                name=f"qTblk{kc}")
            nc.vector.memset(qTblk, 0.0)
            qTblks.append(qTblk)
        # all 16 q matmuls into one PSUM tile first, then all scatters: no
        # per-(kc,h) PE<->ACT ping-pong on a shared buffer
        pq2 = bigP.tile([128, 16, BC], F32, tag="big", name="pq2")
        for kc in range(2):
            for h in range(H):
                idx = kc * H + h
                for kin in range(2):
                    col0 = h * K + kc * 128
                    nc.tensor.matmul(
                        pq2[:, idx, :],
                        lhsT=wmat[:, kin, col0:col0 + 128],
                        rhs=queryT[:, kin, :],
                        start=(kin == 0), stop=(kin == 1),
                        skip_group_check=True,
                    )
        for kc in range(2):
            for h in range(H):
                idx = kc * H + h
                # scatter b -> column 8b+h of batch-b's block (stride 72)
                out_ap = _ap(qTblks[kc], h,
                             [[qTblks[kc].ap[0][0], 128], [72, BC]])
                if kc == 0:
                    nc.scalar.activation(
                        out_ap, pq2[:, idx, :],
                        mybir.ActivationFunctionType.Identity,
                        bias=bq_sb[:, h * 2 + kc:h * 2 + kc + 1], scale=1.0)
                else:
                    nc.vector.tensor_scalar(
                        out=out_ap, in0=pq2[:, idx, :],
                        scalar1=bq_sb[:, h * 2 + kc:h * 2 + kc + 1],
                        scalar2=None, op0=mybir.AluOpType.add)

        # additive -1e30 mask from runtime steps
        addmask = consts.tile([64, L], F32, tag="addmask")
        nc.vector.tensor_scalar(
            out=addmask, in0=iota, scalar1=stepsf, scalar2=NEG,
            op0=mybir.AluOpType.is_ge, op1=mybir.AluOpType.mult)

        # ------------- scores: one [64, 2048] PSUM tile -------------
        # Per chunk: 2*nb matmuls accumulate; rpe modulation is applied on
        # copyback of the live rows; the mask add is fused with a running
        # per-chunk max so everything trails the keys stream and negmax is
        # ready right after the last chunk.
        live_k = [ch for ch in range(NKC) if nbk[ch] > 0]
        runmax = consts.tile([64, NKC], F32, tag="runmax")
        for ch in live_k:
            kt = keys_tiles[ch]
            nb = nbk[ch]
            n_mm = 2 * nb
            i_mm = 0
            pscore = pscP.tile([64, KCL], F32, tag="psc", name="pscore")
            for kc in range(2):
                for b in range(nb):
                    nc.tensor.matmul(
                        pscore,
                        lhsT=qTblks[kc][:, b, :],
                        rhs=kt[:, kc, b, :],
                        start=(i_mm == 0), stop=(i_mm == n_mm - 1))
                    i_mm += 1
            lo = ch * KCL
            nc.vector.tensor_mul(scoresT[:8 * nb, lo:lo + KCL],
                                 pscore[:8 * nb, :],
                                 rpeT[:8 * nb, lo:lo + KCL])
            nc.vector.tensor_add(scoresT[:, lo:lo + KCL],
                                 scoresT[:, lo:lo + KCL],
                                 addmask[:, lo:lo + KCL])
            nc.vector.reduce_max(runmax[:, ch:ch + 1],
                                 scoresT[:, lo:lo + KCL],
                                 axis=mybir.AxisListType.X)

        # ------------- softmax over l (free dim) -------------
        # nbk is non-increasing (batches sorted by steps), so live chunks are
        # a prefix and runmax[:, :n_live] is exactly the written region
        negmax = consts.tile([64, 1], F32, tag="negmax")
        nc.vector.reduce_max(negmax, runmax[:, :len(live_k)],
                             axis=mybir.AxisListType.X, negate=True)
        sumexp = consts.tile([64, 1], F32, tag="sumexp")
        nc.scalar.activation(scoresT, scoresT,
                             mybir.ActivationFunctionType.Exp,
                             bias=negmax, scale=1.0,
                             accum_out=sumexp)
        recip = consts.tile([64, 1], F32, tag="recip")
        nc.vector.reciprocal(recip, sumexp)
        # diag(recip): one regular matmul against it transposes a w chunk AND
        # applies the softmax denominator in the same PE pass (out[l, bh] =
        # sum_r scoresT[r, l] * diag[r, bh] = scoresT[bh, l] * recip[bh])
        dmat = consts.tile([64, 64], F32, tag="dmat")
        nc.vector.tensor_scalar(
            out=dmat, in0=ident[:64, :64], scalar1=recip, scalar2=None,
            op0=mybir.AluOpType.mult)

        # ------------- read: accumulate readT[v, bh] over all l -------------
        # lastvc[b]: the last chunk where batch-slot b is live (per-column
        # accumulation groups need their stop on their own final matmul).
        # The transpose+normalize matmuls (PE) are emitted one chunk ahead of
        # the read matmuls so PE never stalls on the w_sb copyback.
        lastvc = [max(vc for vc in range(NVC) if nbv[vc] > b)
                  for b in range(BC)]
        live_v = [vc for vc in range(NVC) if nbv[vc] > 0]
        preadT = [prP.tile([128, 64], F32, tag="pr", name=f"preadT{vh}")
                  for vh in range(2)]

        pw_tiles = {}

        def emit_wT(vc):
            pw = pwP.tile([128, 64], F32, tag="pw")
            off = vc * VCL
            nc.tensor.matmul(pw, lhsT=scoresT[:, off:off + VCL],
                             rhs=dmat, start=True, stop=True)
            pw_tiles[vc] = pw

        emit_wT(live_v[0])
        if len(live_v) > 1:
            emit_wT(live_v[1])
        for i, vc in enumerate(live_v):
            if i + 2 < len(live_v):
                emit_wT(live_v[i + 2])
            vt = vals_tiles[vc]
            nb = nbv[vc]
            w_sb = wsbP.tile([128, 64], F16, tag="wsb")
            cb = nc.vector.tensor_copy if i % 2 == 0 else nc.scalar.copy
            cb(w_sb, pw_tiles.pop(vc))
            for vh in range(2):
                for b in range(nb):
                    r = rend[b] if vc == lastvc[b] else VCL
                    nc.tensor.matmul(
                        preadT[vh][:, 8 * b:8 * b + 8],
                        lhsT=vt[:r, b, vh * 128:(vh + 1) * 128],
                        rhs=w_sb[:r, 8 * b:8 * b + 8],
                        start=(vc == live_v[0] and b == 0),
                        stop=(vc == lastvc[b]),
                        skip_group_check=True)

        # ------------- epilogue: head aggregation + store -------------
        readT_sb = consts.tile([128, 2, 64], F16, tag="readT_sb")
        nc.vector.tensor_copy(readT_sb[:, 0, :], preadT[0])
        nc.scalar.copy(readT_sb[:, 1, :], preadT[1])

        po = bigP.tile([64, V], F32, tag="big", name="po")
        n_mm = 2 * H
        i_mm = 0
        for h in range(H):      # wa-chunk order h*2+half: 0..15
            for half in range(2):
                lhsT = _ap(readT_sb, half * 64 + h,
                           [[readT_sb.ap[0][0], 128], [8, BC]])
                nc.tensor.matmul(
                    po[:BC, :], lhsT=lhsT, rhs=wa_sb[:, h * 2 + half, :],
                    start=(i_mm == 0), stop=(i_mm == n_mm - 1))
                i_mm += 1
        out_sb = consts.tile([BC, V], F32, tag="out_sb")
        nc.vector.tensor_add(out_sb, po[:BC, :], ba_rep)
        nc.sync.dma_start(out=t_out, in_=out_sb)


_NC_CACHE = {}
_LAST_NC = None


def _get_nc(nbk=None, nbv=None, rend=None):
    global _LAST_NC
    if nbk is None:
        # test/profiling convenience: the program from the latest kernel()
        # call (or the untruncated profile if none was made yet)
        if _LAST_NC is None:
            return _get_nc((BC,) * NKC, (BC,) * NVC, (VCL,) * BC)
        return _LAST_NC
    key = (nbk, nbv, rend)
    if key not in _NC_CACHE:
        _NC_CACHE[key] = build_nc(nbk, nbv, rend)
    _LAST_NC = _NC_CACHE[key]
    return _LAST_NC


def make_in_maps(query, keys, vals, rpe_mod, Wq, bq, Wa, ba, steps):
    core_idx = _plan(steps)[0]
    wq16 = np.ascontiguousarray(Wq, dtype=np.float16)
    wa16 = np.ascontiguousarray(Wa, dtype=np.float16)
    bq32 = np.ascontiguousarray(bq, dtype=np.float32)
    ba32 = np.ascontiguousarray(ba, dtype=np.float32)
    rpe = np.asarray(rpe_mod)[:, :, 0]  # [L, B]
    in_maps = []
    for c in range(NCORES):
        bs = core_idx[c]
        # keysT[ch, kp, kc, b, l] = keys[ch*256 + l, b, kc*128 + kp]
        kc_ = np.asarray(keys[:, bs, :]).reshape(NKC, KCL, BC, 2, 128)
        keysT = np.ascontiguousarray(
            kc_.transpose(0, 4, 3, 2, 1), dtype=np.float16)
        vals_c = np.ascontiguousarray(
            np.asarray(vals[:, bs, :]).reshape(NVC, VCL, BC, V),
            dtype=np.float16)
        rpeT = np.ascontiguousarray(
            np.repeat(rpe[:, bs].T, H, axis=0), dtype=np.float16)
        stepsf = np.repeat(
            np.asarray(steps[bs]).astype(np.float32), H)
        in_maps.append({
            "query": np.ascontiguousarray(query[bs], dtype=np.float32),
            "keysT": keysT,
            "vals": vals_c,
            "rpeT": rpeT,
            "wq": wq16,
            "bq": bq32,
            "wa": wa16,
            "ba": ba32,
            "stepsf": np.ascontiguousarray(stepsf, dtype=np.float32),
        })
    return in_maps


def kernel(query, keys, vals, rpe_mod, Wq, bq, Wa, ba, steps):
    query = np.asarray(query)
    keys = np.asarray(keys)
    vals = np.asarray(vals)
    rpe_mod = np.asarray(rpe_mod)
    Wq = np.asarray(Wq)
    bq = np.asarray(bq)
    Wa = np.asarray(Wa)
    ba = np.asarray(ba)
    steps = np.asarray(steps)

    core_idx, nbk, nbv, rend = _plan(steps)
    nc = _get_nc(nbk, nbv, rend)
    in_maps = make_in_maps(query, keys, vals, rpe_mod, Wq, bq, Wa, ba, steps)
    res = run_bass_kernel_spmd(nc, in_maps, core_ids=list(range(NCORES)))
    out = np.empty((B, V), dtype=np.float32)
    for c in range(NCORES):
        out[core_idx[c]] = res.results[c]["out"].astype(np.float32)
    return out
